# revision 15
# baseline (speedup 1.0000x reference)
"""MoE audio projector kernel for 8 Trainium2 NeuronCores.

Sharding: 8 cores = 4 token groups x 2 weight (HID) halves.
  - Each token group g covers batches [4g, 4g+4) = 1500 stacked tokens.
  - Each half s computes all 5 MLP units (4 experts + shared) restricted to
    HID rows [s*1024, (s+1)*1024); host sums the two partial outputs.
Everything else (frame stacking = free reshape, RMSNorm, router softmax,
top-2 gates, gelu MLPs, gate-weighted combine, biases) runs on device.

Matmuls run in bf16 (full-rate PE mode) accumulating in fp32 PSUM; weights
are pre-laid-out on host so every DMA is 128 partitions x contiguous bytes.
"""
import sys

sys.path.insert(0, "/opt/trn_rl_repo")

import numpy as np
import ml_dtypes

import concourse.bass as bass
import concourse.mybir as mybir
import concourse.tile as tile
from concourse.bass_utils import run_bass_kernel_spmd
from concourse.masks import make_identity

P = 128
IN_DIM = 5120
K1 = IN_DIM // P          # 40 contraction tiles for mm1 / router
HID = 2048
WS = 2                    # weight-split ways (HID halves)
HIDH = HID // WS          # 1024
K2 = HIDH // P            # 8 contraction tiles for mm2
OUT_DIM = 2048
UNITS = 5                 # 4 experts + shared
E = 4
TG = 4                    # token groups
B_PER_G = 16 // TG        # 4 batches per group
T_CORE = 1500             # valid tokens per core
NT = 512                  # token chunk (SBUF-resident)
NCHUNK = 3                # 3 x 512 = 1536 (36 pad tokens)
TT_PER_CHUNK = NT // P    # 4
OSL = 512                 # mm2 output slice
NOSL = OUT_DIM // OSL     # 4
EPS_NORM = 1e-6
EPS_GATE = 1e-6
ONES_ROW = 32             # partition holding the constant-1 gate row

BF16 = mybir.dt.bfloat16
NPBF16 = ml_dtypes.bfloat16


def split_excess_waits(nc, maxw=1):
    """This container's walrus build only accepts one sync-wait command on
    CTRL-class instructions (Drain) and two on regular ones; fan extra waits
    out onto preceding same-engine NoOps."""
    for f in nc.m.functions:
        for b in f.blocks:
            newlist = []
            for inst in b.instructions:
                lim = maxw
                si = inst.sync_info
                if si is not None and si.on_wait and len(si.on_wait) > lim:
                    waits = list(si.on_wait)
                    chunks = [waits[i:i + lim] for i in range(0, len(waits), lim)]
                    for ci, ch in enumerate(chunks[:-1]):
                        d = mybir.InstNoOp(
                            name=f"{inst.name}-waitsplit{ci}",
                            ins=[], outs=[],
                            sync_info=mybir.SyncInfo(on_wait=ch, on_update=[]),
                        )
                        d.engine = inst.engine
                        nc.register_instruction(d)
                        newlist.append(d)
                    si.on_wait = chunks[-1]
                newlist.append(inst)
            b.instructions = newlist


def build_nc():
    nc = bass.Bass()
    x = nc.dram_tensor("x", [T_CORE, IN_DIM], mybir.dt.float32,
                       kind="ExternalInput")
    # host-pre-laid-out weights: every [P, ...] DMA is contiguous per partition
    w1t = nc.dram_tensor("w1t", [UNITS, K2, P, K1 * P], BF16,
                         kind="ExternalInput")
    w2t = nc.dram_tensor("w2t", [UNITS, NOSL, P, K2 * OSL], BF16,
                         kind="ExternalInput")
    rwt = nc.dram_tensor("rwt", [P, K1, E], mybir.dt.float32,
                         kind="ExternalInput")
    normw = nc.dram_tensor("normw", [P, K1], mybir.dt.float32,
                           kind="ExternalInput")
    b1t = nc.dram_tensor("b1t", [P, UNITS, K2], mybir.dt.float32,
                         kind="ExternalInput")
    b2aug = nc.dram_tensor("b2aug", [P, OUT_DIM], BF16, kind="ExternalInput")
    y = nc.dram_tensor("y", [T_CORE, OUT_DIM], mybir.dt.float32,
                       kind="ExternalOutput")

    f32 = mybir.dt.float32

    with tile.TileContext(nc) as tc:
        with (
            tc.tile_pool(name="singles", bufs=1) as singles,
            tc.tile_pool(name="xq", bufs=2) as xq_pool,
            tc.tile_pool(name="sq", bufs=1) as sq_pool,
            tc.tile_pool(name="xnt", bufs=1) as xnt_pool,
            tc.tile_pool(name="ht", bufs=2) as ht_pool,
            tc.tile_pool(name="oacc", bufs=1) as oacc_pool,
            tc.tile_pool(name="w1s", bufs=2) as w1_pool,
            tc.tile_pool(name="xnf", bufs=2) as xnf_pool,
            tc.tile_pool(name="w2s", bufs=2) as w2_pool,
            tc.tile_pool(name="grow", bufs=2) as grow_pool,
            tc.tile_pool(name="gsmall", bufs=3) as gsmall,
            tc.tile_pool(name="psum1", bufs=2, space="PSUM") as psum1_pool,
            tc.tile_pool(name="psum2", bufs=2, space="PSUM") as psum2_pool,
            tc.tile_pool(name="ptr", bufs=2, space="PSUM") as ptr_pool,
            tc.tile_pool(name="psmall", bufs=1, space="PSUM") as psmall_pool,
        ):
            # ---- constants ----
            ident = singles.tile([P, P], f32)
            make_identity(nc, ident)
            eps_sb = singles.tile([P, 1], f32)
            nc.vector.memset(eps_sb, EPS_NORM)
            normw_sb = singles.tile([P, K1], f32)
            nc.sync.dma_start(normw_sb, normw[:, :])
            rwt_sb = singles.tile([P, K1, E], f32)
            nc.sync.dma_start(rwt_sb, rwt[:, :, :])
            b1_sb = singles.tile([P, UNITS, K2], f32)
            nc.sync.dma_start(b1_sb, b1t[:, :, :])
            b2_sb = singles.tile([P, OUT_DIM], BF16)
            nc.sync.dma_start(b2_sb, b2aug[:, :])
            gates_init = singles.tile([P, NT], BF16)
            nc.vector.memset(gates_init, 0.0)
            nc.vector.memset(gates_init[ONES_ROW:ONES_ROW + 1, :], 1.0)

            for c in range(NCHUNK):
                c0 = c * NT
                # chunk-resident K-major normalized tokens [P, K1, NT]
                xnT = xnt_pool.tile([P, K1, NT], BF16, tag="xnt")
                # gate rows for the bias matmul (rows 0..3 = expert gates,
                # row 4 = 1.0 for the shared bias, rest 0)
                gates = grow_pool.tile([P, NT], BF16, tag="gates")
                nc.vector.tensor_copy(gates, gates_init)
                # token-major gates for the combine scaling
                gtok = grow_pool.tile([P, TT_PER_CHUNK, E], f32, tag="gtok")

                # ---------- phase A: rmsnorm + transpose + router ----------
                for tt in range(TT_PER_CHUNK):
                    ts = tt * P
                    tok0 = c0 + ts                    # global token row
                    tlen = min(P, max(0, T_CORE - tok0))
                    xq = xq_pool.tile([P, IN_DIM], f32, tag="xq")
                    if tlen < P:
                        nc.vector.memset(xq, 0.0)
                    if tlen > 0:
                        nc.sync.dma_start(xq[:tlen, :], x[tok0:tok0 + tlen, :])
                    # sum of squares via ACT Square with free-accum
                    ssqs = gsmall.tile([P, 10], f32, tag="ssqs")
                    for q in range(10):
                        sq = sq_pool.tile([P, IN_DIM // 10], BF16, tag="sq")
                        nc.scalar.activation(
                            out=sq, in_=xq[:, q * 512:(q + 1) * 512],
                            func=mybir.ActivationFunctionType.Square,
                            accum_out=ssqs[:, q:q + 1],
                        )
                    ssq = gsmall.tile([P, 1], f32, tag="ssq")
                    nc.vector.reduce_sum(ssq, ssqs, axis=mybir.AxisListType.X)
                    # rstd = 1/sqrt(ssq/IN_DIM + eps)
                    rstd = gsmall.tile([P, 1], f32, tag="rstd")
                    nc.scalar.activation(
                        out=rstd, in_=ssq,
                        func=mybir.ActivationFunctionType.Sqrt,
                        bias=eps_sb, scale=1.0 / IN_DIM,
                    )
                    nc.vector.reciprocal(rstd, rstd)
                    # diag(rstd) so the transpose matmul applies the scale
                    diag = gsmall.tile([P, P], f32, tag="diag")
                    nc.gpsimd.affine_select(
                        out=diag, in_=rstd.to_broadcast((P, P)),
                        pattern=[[-1, P]], channel_multiplier=1, base=0,
                        compare_op=mybir.AluOpType.is_equal, fill=0.0,
                    )
                    # transpose: xn = norm_w * (xq.T @ diag), staged in f32
                    # for exact router logits, cast to bf16 for the MLPs
                    pr_l = psmall_pool.tile([P, E], f32, tag="ps")
                    for kb in range(K1 // 4):
                        ptr = ptr_pool.tile([P, 4, P], f32, tag="ptr")
                        for kq in range(4):
                            kg = kb * 4 + kq
                            nc.tensor.matmul(
                                ptr[:, kq, :],
                                lhsT=xq[:, kg * P:(kg + 1) * P],
                                rhs=diag,
                                start=True, stop=True,
                            )
                        xnf = xnf_pool.tile([P, 4, P], f32, tag="xnf")
                        nc.vector.tensor_mul(
                            xnf, ptr,
                            normw_sb[:, kb * 4:(kb + 1) * 4].to_broadcast(
                                (P, 4, P)),
                        )
                        nc.scalar.activation(
                            out=xnT[:, kb * 4:(kb + 1) * 4, ts:ts + P],
                            in_=xnf,
                            func=mybir.ActivationFunctionType.Copy)
                        for kq in range(4):
                            kg = kb * 4 + kq
                            nc.tensor.matmul(
                                pr_l,
                                lhsT=xnf[:, kq, :],
                                rhs=rwt_sb[:, kg, :],
                                start=(kg == 0), stop=(kg == K1 - 1),
                            )
                    lg = gsmall.tile([P, E], f32, tag="lg")
                    nc.vector.tensor_copy(lg, pr_l)
                    # softmax over E (free dim)
                    mx = gsmall.tile([P, 1], f32, tag="mx")
                    nc.vector.reduce_max(mx, lg, axis=mybir.AxisListType.X)
                    nmx = gsmall.tile([P, 1], f32, tag="nmx")
                    nc.scalar.mul(nmx, mx, -1.0)
                    pr = gsmall.tile([P, E], f32, tag="pr")
                    nc.scalar.activation(
                        out=pr, in_=lg, func=mybir.ActivationFunctionType.Exp,
                        bias=nmx, scale=1.0,
                    )
                    ssum = gsmall.tile([P, 1], f32, tag="ssum")
                    nc.vector.reduce_sum(ssum, pr, axis=mybir.AxisListType.X)
                    rsum = gsmall.tile([P, 1], f32, tag="rsum")
                    nc.vector.reciprocal(rsum, ssum)
                    nc.vector.tensor_scalar_mul(pr, pr, rsum)
                    # top-2 membership + renormalize
                    m1 = gsmall.tile([P, 1], f32, tag="m1")
                    nc.vector.reduce_max(m1, pr, axis=mybir.AxisListType.X)
                    mk = gsmall.tile([P, E], f32, tag="mk")
                    nc.vector.tensor_scalar(
                        out=mk, in0=pr, scalar1=m1, scalar2=None,
                        op0=mybir.AluOpType.is_equal,
                    )
                    mskd = gsmall.tile([P, E], f32, tag="mskd")
                    nc.vector.tensor_scalar(
                        out=mskd, in0=mk, scalar1=-2.0, scalar2=None,
                        op0=mybir.AluOpType.mult,
                    )
                    nc.vector.tensor_add(mskd, mskd, pr)
                    m2 = gsmall.tile([P, 1], f32, tag="m2")
                    nc.vector.reduce_max(m2, mskd, axis=mybir.AxisListType.X)
                    den = gsmall.tile([P, 1], f32, tag="den")
                    nc.vector.tensor_add(den, m1, m2)
                    nc.vector.tensor_scalar_add(den, den, EPS_GATE)
                    rden = gsmall.tile([P, 1], f32, tag="rden")
                    nc.vector.reciprocal(rden, den)
                    sel = gsmall.tile([P, E], f32, tag="sel")
                    nc.vector.tensor_scalar(
                        out=sel, in0=pr, scalar1=m2, scalar2=None,
                        op0=mybir.AluOpType.is_ge,
                    )
                    nc.vector.tensor_mul(gtok[:, tt, :], pr, sel)
                    nc.vector.tensor_scalar_mul(gtok[:, tt, :], gtok[:, tt, :],
                                                rden)
                    # transpose gates to expert-major rows [E, P] for bias MM
                    pgt = psmall_pool.tile([P, P], f32, tag="pgt")
                    nc.tensor.transpose(pgt[:E, :], gtok[:, tt, :], ident)
                    nc.vector.tensor_copy(gates[0:E, ts:ts + P], pgt[:E, :])

                # ---------- phase B: expert MLPs ----------
                oacc = oacc_pool.tile([P, TT_PER_CHUNK, OUT_DIM], f32,
                                      tag="oacc")
                for u in range(UNITS):
                    hT = ht_pool.tile([P, K2, NT], BF16, tag="ht")
                    for h in range(K2):
                        w1s = w1_pool.tile([P, K1, P], BF16, tag="w1s")
                        nc.sync.dma_start(
                            w1s, w1t[u, h].rearrange("p (k f) -> p k f", f=P))
                        ps1 = psum1_pool.tile([P, NT], f32, tag="ps1")
                        for k in range(K1):
                            nc.tensor.matmul(
                                ps1,
                                lhsT=w1s[:, k, :],
                                rhs=xnT[:, k, :],
                                start=(k == 0), stop=(k == K1 - 1),
                            )
                        nc.scalar.activation(
                            out=hT[:, h, :], in_=ps1,
                            func=mybir.ActivationFunctionType.Gelu,
                            bias=b1_sb[:, u, h:h + 1], scale=1.0,
                        )
                    for osl in range(NOSL):
                        w2s = w2_pool.tile([P, K2, OSL], BF16, tag="w2s")
                        nc.sync.dma_start(
                            w2s, w2t[u, osl].rearrange("p (k f) -> p k f",
                                                       f=OSL))
                        osl_sl = slice(osl * OSL, (osl + 1) * OSL)
                        for tt in range(TT_PER_CHUNK):
                            ts = tt * P
                            ps2 = psum2_pool.tile([P, OSL], f32, tag="ps2")
                            for k in range(K2):
                                nc.tensor.matmul(
                                    ps2,
                                    lhsT=hT[:, k, ts:ts + P],
                                    rhs=w2s[:, k, :],
                                    start=(k == 0),
                                    stop=(k == K2 - 1 and u != UNITS - 1),
                                )
                            if u < E:
                                # scale by this expert's gate (per token row)
                                nc.vector.tensor_scalar_mul(
                                    ps2, ps2, gtok[:, tt, u:u + 1]
                                )
                                if u == 0:
                                    nc.vector.tensor_copy(
                                        oacc[:, tt, osl_sl], ps2
                                    )
                                else:
                                    nc.vector.tensor_add(
                                        oacc[:, tt, osl_sl],
                                        oacc[:, tt, osl_sl], ps2,
                                    )
                            else:
                                # shared unit: fold gate-weighted b2 + sb2
                                nc.tensor.matmul(
                                    ps2,
                                    lhsT=gates[:, ts:ts + P],
                                    rhs=b2_sb[:, osl_sl],
                                    start=False, stop=True,
                                )
                                nc.vector.tensor_add(
                                    oacc[:, tt, osl_sl],
                                    oacc[:, tt, osl_sl], ps2,
                                )
                # ---------- output ----------
                for tt in range(TT_PER_CHUNK):
                    tok0 = c0 + tt * P
                    tlen = min(P, max(0, T_CORE - tok0))
                    if tlen > 0:
                        nc.sync.dma_start(
                            y[tok0:tok0 + tlen, :], oacc[:tlen, tt, :]
                        )

    split_excess_waits(nc)
    return nc


_NC_CACHE = {}


def _get_nc():
    if "nc" not in _NC_CACHE:
        _NC_CACHE["nc"] = build_nc()
    return _NC_CACHE["nc"]


def build_in_maps(x, norm_w, router_w, w1, b1, w2, b2, sw1, sb1, sw2, sb2):
    x = np.ascontiguousarray(np.asarray(x, dtype=np.float32))
    norm_w = np.asarray(norm_w, dtype=np.float32)
    router_w = np.asarray(router_w, dtype=np.float32)

    Bsz, S, D = x.shape          # [16, 1500, 1280]
    out_len = (S - 4) // 4 + 1   # 375
    # frame stacking is a free reshape: [B, S, D] -> [B, out_len, 4*D]
    xs = x[:, :out_len * 4, :].reshape(Bsz, out_len, 4 * D)

    # stack 5 units: 4 experts + shared
    w1_all = np.concatenate([w1, sw1[None]], axis=0)       # [5, HID, IN]
    b1_all = np.concatenate([b1, sb1[None]], axis=0)       # [5, HID]
    w2_all = np.concatenate([w2, sw2[None]], axis=0)       # [5, OUT, HID]

    # router weights K-major: [P, K1, E]
    rwt_np = np.ascontiguousarray(
        router_w.T.reshape(K1, P, E).transpose(1, 0, 2))
    normw_np = np.ascontiguousarray(norm_w.reshape(K1, P).T)
    b2aug_np = np.zeros((P, OUT_DIM), dtype=np.float32)
    b2aug_np[:E, :] = b2 / WS
    b2aug_np[ONES_ROW, :] = sb2 / WS
    b2aug_np = b2aug_np.astype(NPBF16)

    half_maps = []
    for s in range(WS):
        hsl = slice(s * HIDH, (s + 1) * HIDH)
        w1h = w1_all[:, hsl, :]                            # [5, HIDH, IN]
        # [u, h, pi, po, f] = w1h[u, h*P+f, po*P+pi]
        w1t_np = np.ascontiguousarray(
            w1h.reshape(UNITS, K2, P, K1, P).transpose(0, 1, 4, 3, 2)
            .reshape(UNITS, K2, P, K1 * P)).astype(NPBF16)
        w2h = w2_all[:, :, hsl]                            # [5, OUT, HIDH]
        # [u, osl, pi, po, f] = w2h[u, osl*OSL+f, po*P+pi]
        w2t_np = np.ascontiguousarray(
            w2h.reshape(UNITS, NOSL, OSL, K2, P).transpose(0, 1, 4, 3, 2)
            .reshape(UNITS, NOSL, P, K2 * OSL)).astype(NPBF16)
        b1t_np = np.ascontiguousarray(
            b1_all[:, hsl].reshape(UNITS, K2, P).transpose(2, 0, 1))
        half_maps.append({
            "w1t": w1t_np, "w2t": w2t_np, "b1t": b1t_np,
            "rwt": rwt_np, "normw": normw_np, "b2aug": b2aug_np,
        })

    in_maps = []
    for core in range(8):
        g, s = divmod(core, WS)
        xg = np.ascontiguousarray(
            xs[g * B_PER_G:(g + 1) * B_PER_G].reshape(T_CORE, IN_DIM))
        m = dict(half_maps[s])
        m["x"] = xg
        in_maps.append(m)
    return in_maps


def kernel(x, norm_w, router_w, w1, b1, w2, b2, sw1, sb1, sw2, sb2):
    in_maps = build_in_maps(x, norm_w, router_w, w1, b1, w2, b2,
                            sw1, sb1, sw2, sb2)
    nc = _get_nc()
    # the axon-proxied execute occasionally hits a transient
    # NRT_EXEC_UNIT_UNRECOVERABLE; retry a few times
    last_exc = None
    for attempt in range(4):
        try:
            res = run_bass_kernel_spmd(nc, in_maps, core_ids=list(range(8)))
            break
        except Exception as exc:  # noqa: BLE001
            last_exc = exc
            import time
            time.sleep(5)
    else:
        raise last_exc

    Bsz = 16
    out_len = 375
    out = np.empty((Bsz, out_len, OUT_DIM), dtype=np.float32)
    for g in range(TG):
        part = res.results[g * WS]["y"] + res.results[g * WS + 1]["y"]
        out[g * B_PER_G:(g + 1) * B_PER_G] = part.reshape(
            B_PER_G, out_len, OUT_DIM)
    return out


# revision 17
# speedup vs baseline: 1.0098x; 1.0098x over previous
"""MoE audio projector kernel for 8 Trainium2 NeuronCores.

Sharding: 8 cores = 4 token groups x 2 weight (HID) halves.
  - Each token group g covers batches [4g, 4g+4) = 1500 stacked tokens.
  - Each half s computes all 5 MLP units (4 experts + shared) restricted to
    HID rows [s*1024, (s+1)*1024); host sums the two partial outputs.
Everything else (frame stacking = free reshape, RMSNorm, router softmax,
top-2 gates, gelu MLPs, gate-weighted combine, biases) runs on device.

Matmuls run in bf16 (full-rate PE mode) accumulating in fp32 PSUM; weights
are pre-laid-out on host so every DMA is 128 partitions x contiguous bytes.
"""
import sys

sys.path.insert(0, "/opt/trn_rl_repo")

import numpy as np
import ml_dtypes

import concourse.bass as bass
import concourse.mybir as mybir
import concourse.tile as tile
from concourse.bass_utils import run_bass_kernel_spmd
from concourse.masks import make_identity

P = 128
IN_DIM = 5120
K1 = IN_DIM // P          # 40 contraction tiles for mm1 / router
HID = 2048
WS = 2                    # weight-split ways (HID halves)
HIDH = HID // WS          # 1024
K2 = HIDH // P            # 8 contraction tiles for mm2
OUT_DIM = 2048
UNITS = 5                 # 4 experts + shared
E = 4
TG = 4                    # token groups
B_PER_G = 16 // TG        # 4 batches per group
T_CORE = 1500             # valid tokens per core
NT = 512                  # token chunk (SBUF-resident)
NCHUNK = 3                # 3 x 512 = 1536 (36 pad tokens)
TT_PER_CHUNK = NT // P    # 4
OSL = 512                 # mm2 output slice
NOSL = OUT_DIM // OSL     # 4
EPS_NORM = 1e-6
EPS_GATE = 1e-6
ONES_ROW = 32             # partition holding the constant-1 gate row

BF16 = mybir.dt.bfloat16
NPBF16 = ml_dtypes.bfloat16


def split_excess_waits(nc, maxw=1):
    """This container's walrus build only accepts one sync-wait command on
    CTRL-class instructions (Drain) and two on regular ones; fan extra waits
    out onto preceding same-engine NoOps."""
    for f in nc.m.functions:
        for b in f.blocks:
            newlist = []
            for inst in b.instructions:
                lim = maxw
                si = inst.sync_info
                if si is not None and si.on_wait and len(si.on_wait) > lim:
                    waits = list(si.on_wait)
                    chunks = [waits[i:i + lim] for i in range(0, len(waits), lim)]
                    for ci, ch in enumerate(chunks[:-1]):
                        d = mybir.InstNoOp(
                            name=f"{inst.name}-waitsplit{ci}",
                            ins=[], outs=[],
                            sync_info=mybir.SyncInfo(on_wait=ch, on_update=[]),
                        )
                        d.engine = inst.engine
                        nc.register_instruction(d)
                        newlist.append(d)
                    si.on_wait = chunks[-1]
                newlist.append(inst)
            b.instructions = newlist


def build_nc():
    nc = bass.Bass()
    x = nc.dram_tensor("x", [T_CORE, IN_DIM], mybir.dt.float32,
                       kind="ExternalInput")
    # host-pre-laid-out weights: every [P, ...] DMA is contiguous per partition
    w1t = nc.dram_tensor("w1t", [UNITS, K2, P, K1 * P], BF16,
                         kind="ExternalInput")
    w2t = nc.dram_tensor("w2t", [UNITS, NOSL, P, K2 * OSL], BF16,
                         kind="ExternalInput")
    rwt = nc.dram_tensor("rwt", [P, K1, E], mybir.dt.float32,
                         kind="ExternalInput")
    normw = nc.dram_tensor("normw", [P, K1], mybir.dt.float32,
                           kind="ExternalInput")
    b1t = nc.dram_tensor("b1t", [P, UNITS, K2], mybir.dt.float32,
                         kind="ExternalInput")
    b2aug = nc.dram_tensor("b2aug", [P, OUT_DIM], BF16, kind="ExternalInput")
    y = nc.dram_tensor("y", [T_CORE, OUT_DIM], mybir.dt.float32,
                       kind="ExternalOutput")

    f32 = mybir.dt.float32

    with tile.TileContext(nc) as tc:
        with (
            tc.tile_pool(name="singles", bufs=1) as singles,
            tc.tile_pool(name="xq", bufs=2) as xq_pool,
            tc.tile_pool(name="sq", bufs=1) as sq_pool,
            tc.tile_pool(name="xnt", bufs=1) as xnt_pool,
            tc.tile_pool(name="ht", bufs=2) as ht_pool,
            tc.tile_pool(name="oacc", bufs=1) as oacc_pool,
            tc.tile_pool(name="w1s", bufs=2) as w1_pool,
            tc.tile_pool(name="xnf", bufs=2) as xnf_pool,
            tc.tile_pool(name="w2s", bufs=2) as w2_pool,
            tc.tile_pool(name="grow", bufs=2) as grow_pool,
            tc.tile_pool(name="gsmall", bufs=3) as gsmall,
            tc.tile_pool(name="psum1", bufs=2, space="PSUM") as psum1_pool,
            tc.tile_pool(name="psum2", bufs=2, space="PSUM") as psum2_pool,
            tc.tile_pool(name="ptr", bufs=2, space="PSUM") as ptr_pool,
            tc.tile_pool(name="psmall", bufs=1, space="PSUM") as psmall_pool,
        ):
            # ---- constants ----
            ident = singles.tile([P, P], f32)
            make_identity(nc, ident)
            eps_sb = singles.tile([P, 1], f32)
            nc.vector.memset(eps_sb, EPS_NORM)
            normw_sb = singles.tile([P, K1], f32)
            nc.sync.dma_start(normw_sb, normw[:, :])
            rwt_sb = singles.tile([P, K1, E], f32)
            nc.sync.dma_start(rwt_sb, rwt[:, :, :])
            b1_sb = singles.tile([P, UNITS, K2], f32)
            nc.sync.dma_start(b1_sb, b1t[:, :, :])
            b2_sb = singles.tile([P, OUT_DIM], BF16)
            nc.sync.dma_start(b2_sb, b2aug[:, :])
            gates_init = singles.tile([P, NT], BF16)
            nc.vector.memset(gates_init, 0.0)
            nc.vector.memset(gates_init[ONES_ROW:ONES_ROW + 1, :], 1.0)
            # warm-up: keep the PE busy through the initial weight-DMA wait
            # so the HAM clock gate opens before the first real matmul
            for wi in range(96):
                pwarm = psum1_pool.tile([P, NT], f32, tag="ps1", name="pwarm")
                nc.tensor.matmul(
                    pwarm, lhsT=gates_init[:, :P], rhs=gates_init,
                    start=True, stop=True,
                )

            for c in range(NCHUNK):
                c0 = c * NT
                # chunk-resident K-major normalized tokens [P, K1, NT]
                xnT = xnt_pool.tile([P, K1, NT], BF16, tag="xnt")
                # gate rows for the bias matmul (rows 0..3 = expert gates,
                # row 4 = 1.0 for the shared bias, rest 0)
                gates = grow_pool.tile([P, NT], BF16, tag="gates")
                nc.vector.tensor_copy(gates, gates_init)
                # token-major gates for the combine scaling
                gtok = grow_pool.tile([P, TT_PER_CHUNK, E], f32, tag="gtok")

                # ---------- phase A: rmsnorm + transpose + router ----------
                for tt in range(TT_PER_CHUNK):
                    ts = tt * P
                    tok0 = c0 + ts                    # global token row
                    tlen = min(P, max(0, T_CORE - tok0))
                    xq = xq_pool.tile([P, IN_DIM], f32, tag="xq")
                    if tlen < P:
                        nc.vector.memset(xq, 0.0)
                    if tlen > 0:
                        nc.sync.dma_start(xq[:tlen, :], x[tok0:tok0 + tlen, :])
                    # sum of squares via ACT Square with free-accum
                    ssqs = gsmall.tile([P, 10], f32, tag="ssqs")
                    for q in range(10):
                        sq = sq_pool.tile([P, IN_DIM // 10], BF16, tag="sq")
                        nc.scalar.activation(
                            out=sq, in_=xq[:, q * 512:(q + 1) * 512],
                            func=mybir.ActivationFunctionType.Square,
                            accum_out=ssqs[:, q:q + 1],
                        )
                    ssq = gsmall.tile([P, 1], f32, tag="ssq")
                    nc.vector.reduce_sum(ssq, ssqs, axis=mybir.AxisListType.X)
                    # rstd = 1/sqrt(ssq/IN_DIM + eps)
                    rstd = gsmall.tile([P, 1], f32, tag="rstd")
                    nc.scalar.activation(
                        out=rstd, in_=ssq,
                        func=mybir.ActivationFunctionType.Sqrt,
                        bias=eps_sb, scale=1.0 / IN_DIM,
                    )
                    nc.vector.reciprocal(rstd, rstd)
                    # diag(rstd) so the transpose matmul applies the scale
                    diag = gsmall.tile([P, P], f32, tag="diag")
                    nc.gpsimd.affine_select(
                        out=diag, in_=rstd.to_broadcast((P, P)),
                        pattern=[[-1, P]], channel_multiplier=1, base=0,
                        compare_op=mybir.AluOpType.is_equal, fill=0.0,
                    )
                    # transpose: xn = norm_w * (xq.T @ diag), staged in f32
                    # for exact router logits, cast to bf16 for the MLPs
                    pr_l = psmall_pool.tile([P, E], f32, tag="ps")
                    for kb in range(K1 // 4):
                        ptr = ptr_pool.tile([P, 4, P], f32, tag="ptr")
                        for kq in range(4):
                            kg = kb * 4 + kq
                            nc.tensor.matmul(
                                ptr[:, kq, :],
                                lhsT=xq[:, kg * P:(kg + 1) * P],
                                rhs=diag,
                                start=True, stop=True,
                            )
                        xnf = xnf_pool.tile([P, 4, P], f32, tag="xnf")
                        nc.vector.tensor_mul(
                            xnf, ptr,
                            normw_sb[:, kb * 4:(kb + 1) * 4].to_broadcast(
                                (P, 4, P)),
                        )
                        nc.scalar.activation(
                            out=xnT[:, kb * 4:(kb + 1) * 4, ts:ts + P],
                            in_=xnf,
                            func=mybir.ActivationFunctionType.Copy)
                        for kq in range(4):
                            kg = kb * 4 + kq
                            nc.tensor.matmul(
                                pr_l,
                                lhsT=xnf[:, kq, :],
                                rhs=rwt_sb[:, kg, :],
                                start=(kg == 0), stop=(kg == K1 - 1),
                            )
                    lg = gsmall.tile([P, E], f32, tag="lg")
                    nc.vector.tensor_copy(lg, pr_l)
                    # softmax over E (free dim)
                    mx = gsmall.tile([P, 1], f32, tag="mx")
                    nc.vector.reduce_max(mx, lg, axis=mybir.AxisListType.X)
                    nmx = gsmall.tile([P, 1], f32, tag="nmx")
                    nc.scalar.mul(nmx, mx, -1.0)
                    pr = gsmall.tile([P, E], f32, tag="pr")
                    nc.scalar.activation(
                        out=pr, in_=lg, func=mybir.ActivationFunctionType.Exp,
                        bias=nmx, scale=1.0,
                    )
                    ssum = gsmall.tile([P, 1], f32, tag="ssum")
                    nc.vector.reduce_sum(ssum, pr, axis=mybir.AxisListType.X)
                    rsum = gsmall.tile([P, 1], f32, tag="rsum")
                    nc.vector.reciprocal(rsum, ssum)
                    nc.vector.tensor_scalar_mul(pr, pr, rsum)
                    # top-2 membership + renormalize
                    m1 = gsmall.tile([P, 1], f32, tag="m1")
                    nc.vector.reduce_max(m1, pr, axis=mybir.AxisListType.X)
                    mk = gsmall.tile([P, E], f32, tag="mk")
                    nc.vector.tensor_scalar(
                        out=mk, in0=pr, scalar1=m1, scalar2=None,
                        op0=mybir.AluOpType.is_equal,
                    )
                    mskd = gsmall.tile([P, E], f32, tag="mskd")
                    nc.vector.tensor_scalar(
                        out=mskd, in0=mk, scalar1=-2.0, scalar2=None,
                        op0=mybir.AluOpType.mult,
                    )
                    nc.vector.tensor_add(mskd, mskd, pr)
                    m2 = gsmall.tile([P, 1], f32, tag="m2")
                    nc.vector.reduce_max(m2, mskd, axis=mybir.AxisListType.X)
                    den = gsmall.tile([P, 1], f32, tag="den")
                    nc.vector.tensor_add(den, m1, m2)
                    nc.vector.tensor_scalar_add(den, den, EPS_GATE)
                    rden = gsmall.tile([P, 1], f32, tag="rden")
                    nc.vector.reciprocal(rden, den)
                    sel = gsmall.tile([P, E], f32, tag="sel")
                    nc.vector.tensor_scalar(
                        out=sel, in0=pr, scalar1=m2, scalar2=None,
                        op0=mybir.AluOpType.is_ge,
                    )
                    nc.vector.tensor_mul(gtok[:, tt, :], pr, sel)
                    nc.vector.tensor_scalar_mul(gtok[:, tt, :], gtok[:, tt, :],
                                                rden)
                    # transpose gates to expert-major rows [E, P] for bias MM
                    pgt = psmall_pool.tile([P, P], f32, tag="pgt")
                    nc.tensor.transpose(pgt[:E, :], gtok[:, tt, :], ident)
                    nc.vector.tensor_copy(gates[0:E, ts:ts + P], pgt[:E, :])

                # ---------- phase B: expert MLPs ----------
                oacc = oacc_pool.tile([P, TT_PER_CHUNK, OUT_DIM], f32,
                                      tag="oacc")
                for u in range(UNITS):
                    hT = ht_pool.tile([P, K2, NT], BF16, tag="ht")
                    for h in range(K2):
                        w1s = w1_pool.tile([P, K1, P], BF16, tag="w1s")
                        nc.sync.dma_start(
                            w1s, w1t[u, h].rearrange("p (k f) -> p k f", f=P))
                        ps1 = psum1_pool.tile([P, NT], f32, tag="ps1")
                        for k in range(K1):
                            nc.tensor.matmul(
                                ps1,
                                lhsT=w1s[:, k, :],
                                rhs=xnT[:, k, :],
                                start=(k == 0), stop=(k == K1 - 1),
                            )
                        nc.scalar.activation(
                            out=hT[:, h, :], in_=ps1,
                            func=mybir.ActivationFunctionType.Gelu,
                            bias=b1_sb[:, u, h:h + 1], scale=1.0,
                        )
                    for osl in range(NOSL):
                        w2s = w2_pool.tile([P, K2, OSL], BF16, tag="w2s")
                        nc.sync.dma_start(
                            w2s, w2t[u, osl].rearrange("p (k f) -> p k f",
                                                       f=OSL))
                        osl_sl = slice(osl * OSL, (osl + 1) * OSL)
                        for tt in range(TT_PER_CHUNK):
                            ts = tt * P
                            ps2 = psum2_pool.tile([P, OSL], f32, tag="ps2")
                            for k in range(K2):
                                nc.tensor.matmul(
                                    ps2,
                                    lhsT=hT[:, k, ts:ts + P],
                                    rhs=w2s[:, k, :],
                                    start=(k == 0),
                                    stop=(k == K2 - 1 and u != UNITS - 1),
                                )
                            if u < E:
                                # scale by this expert's gate (per token row)
                                nc.vector.tensor_scalar_mul(
                                    ps2, ps2, gtok[:, tt, u:u + 1]
                                )
                                if u == 0:
                                    nc.vector.tensor_copy(
                                        oacc[:, tt, osl_sl], ps2
                                    )
                                else:
                                    nc.vector.tensor_add(
                                        oacc[:, tt, osl_sl],
                                        oacc[:, tt, osl_sl], ps2,
                                    )
                            else:
                                # shared unit: fold gate-weighted b2 + sb2
                                nc.tensor.matmul(
                                    ps2,
                                    lhsT=gates[:, ts:ts + P],
                                    rhs=b2_sb[:, osl_sl],
                                    start=False, stop=True,
                                )
                                nc.vector.tensor_add(
                                    oacc[:, tt, osl_sl],
                                    oacc[:, tt, osl_sl], ps2,
                                )
                # ---------- output ----------
                for tt in range(TT_PER_CHUNK):
                    tok0 = c0 + tt * P
                    tlen = min(P, max(0, T_CORE - tok0))
                    if tlen > 0:
                        nc.sync.dma_start(
                            y[tok0:tok0 + tlen, :], oacc[:tlen, tt, :]
                        )

    split_excess_waits(nc)
    return nc


_NC_CACHE = {}


def _get_nc():
    if "nc" not in _NC_CACHE:
        _NC_CACHE["nc"] = build_nc()
    return _NC_CACHE["nc"]


def build_in_maps(x, norm_w, router_w, w1, b1, w2, b2, sw1, sb1, sw2, sb2):
    x = np.ascontiguousarray(np.asarray(x, dtype=np.float32))
    norm_w = np.asarray(norm_w, dtype=np.float32)
    router_w = np.asarray(router_w, dtype=np.float32)

    Bsz, S, D = x.shape          # [16, 1500, 1280]
    out_len = (S - 4) // 4 + 1   # 375
    # frame stacking is a free reshape: [B, S, D] -> [B, out_len, 4*D]
    xs = x[:, :out_len * 4, :].reshape(Bsz, out_len, 4 * D)

    # stack 5 units: 4 experts + shared
    w1_all = np.concatenate([w1, sw1[None]], axis=0)       # [5, HID, IN]
    b1_all = np.concatenate([b1, sb1[None]], axis=0)       # [5, HID]
    w2_all = np.concatenate([w2, sw2[None]], axis=0)       # [5, OUT, HID]

    # router weights K-major: [P, K1, E]
    rwt_np = np.ascontiguousarray(
        router_w.T.reshape(K1, P, E).transpose(1, 0, 2))
    normw_np = np.ascontiguousarray(norm_w.reshape(K1, P).T)
    b2aug_np = np.zeros((P, OUT_DIM), dtype=np.float32)
    b2aug_np[:E, :] = b2 / WS
    b2aug_np[ONES_ROW, :] = sb2 / WS
    b2aug_np = b2aug_np.astype(NPBF16)

    half_maps = []
    for s in range(WS):
        hsl = slice(s * HIDH, (s + 1) * HIDH)
        w1h = w1_all[:, hsl, :]                            # [5, HIDH, IN]
        # [u, h, pi, po, f] = w1h[u, h*P+f, po*P+pi]
        w1t_np = np.ascontiguousarray(
            w1h.reshape(UNITS, K2, P, K1, P).transpose(0, 1, 4, 3, 2)
            .reshape(UNITS, K2, P, K1 * P)).astype(NPBF16)
        w2h = w2_all[:, :, hsl]                            # [5, OUT, HIDH]
        # [u, osl, pi, po, f] = w2h[u, osl*OSL+f, po*P+pi]
        w2t_np = np.ascontiguousarray(
            w2h.reshape(UNITS, NOSL, OSL, K2, P).transpose(0, 1, 4, 3, 2)
            .reshape(UNITS, NOSL, P, K2 * OSL)).astype(NPBF16)
        b1t_np = np.ascontiguousarray(
            b1_all[:, hsl].reshape(UNITS, K2, P).transpose(2, 0, 1))
        half_maps.append({
            "w1t": w1t_np, "w2t": w2t_np, "b1t": b1t_np,
            "rwt": rwt_np, "normw": normw_np, "b2aug": b2aug_np,
        })

    in_maps = []
    for core in range(8):
        g, s = divmod(core, WS)
        xg = np.ascontiguousarray(
            xs[g * B_PER_G:(g + 1) * B_PER_G].reshape(T_CORE, IN_DIM))
        m = dict(half_maps[s])
        m["x"] = xg
        in_maps.append(m)
    return in_maps


def kernel(x, norm_w, router_w, w1, b1, w2, b2, sw1, sb1, sw2, sb2):
    in_maps = build_in_maps(x, norm_w, router_w, w1, b1, w2, b2,
                            sw1, sb1, sw2, sb2)
    nc = _get_nc()
    # the axon-proxied execute occasionally hits a transient
    # NRT_EXEC_UNIT_UNRECOVERABLE; retry a few times
    last_exc = None
    for attempt in range(4):
        try:
            res = run_bass_kernel_spmd(nc, in_maps, core_ids=list(range(8)))
            break
        except Exception as exc:  # noqa: BLE001
            last_exc = exc
            import time
            time.sleep(5)
    else:
        raise last_exc

    Bsz = 16
    out_len = 375
    out = np.empty((Bsz, out_len, OUT_DIM), dtype=np.float32)
    for g in range(TG):
        part = res.results[g * WS]["y"] + res.results[g * WS + 1]["y"]
        out[g * B_PER_G:(g + 1) * B_PER_G] = part.reshape(
            B_PER_G, out_len, OUT_DIM)
    return out


# revision 18
# speedup vs baseline: 1.0122x; 1.0024x over previous
"""MoE audio projector kernel for 8 Trainium2 NeuronCores.

Sharding: 8 cores = 4 token groups x 2 weight (HID) halves.
  - Each token group g covers batches [4g, 4g+4) = 1500 stacked tokens.
  - Each half s computes all 5 MLP units (4 experts + shared) restricted to
    HID rows [s*1024, (s+1)*1024); host sums the two partial outputs.
Everything else (frame stacking = free reshape, RMSNorm, router softmax,
top-2 gates, gelu MLPs, gate-weighted combine, biases) runs on device.

Matmuls run in bf16 (full-rate PE mode) accumulating in fp32 PSUM; weights
are pre-laid-out on host so every DMA is 128 partitions x contiguous bytes.
"""
import sys

sys.path.insert(0, "/opt/trn_rl_repo")

import numpy as np
import ml_dtypes

import concourse.bass as bass
import concourse.mybir as mybir
import concourse.tile as tile
from concourse.bass_utils import run_bass_kernel_spmd
from concourse.masks import make_identity

P = 128
IN_DIM = 5120
K1 = IN_DIM // P          # 40 contraction tiles for mm1 / router
HID = 2048
WS = 2                    # weight-split ways (HID halves)
HIDH = HID // WS          # 1024
K2 = HIDH // P            # 8 contraction tiles for mm2
OUT_DIM = 2048
UNITS = 5                 # 4 experts + shared
E = 4
TG = 4                    # token groups
B_PER_G = 16 // TG        # 4 batches per group
T_CORE = 1500             # valid tokens per core
NT = 512                  # token chunk (SBUF-resident)
NCHUNK = 3                # 3 x 512 = 1536 (36 pad tokens)
TT_PER_CHUNK = NT // P    # 4
OSL = 512                 # mm2 output slice
NOSL = OUT_DIM // OSL     # 4
EPS_NORM = 1e-6
EPS_GATE = 1e-6
ONES_ROW = 32             # partition holding the constant-1 gate row

BF16 = mybir.dt.bfloat16
NPBF16 = ml_dtypes.bfloat16


def split_excess_waits(nc, maxw=1):
    """This container's walrus build only accepts one sync-wait command on
    CTRL-class instructions (Drain) and two on regular ones; fan extra waits
    out onto preceding same-engine NoOps."""
    for f in nc.m.functions:
        for b in f.blocks:
            newlist = []
            for inst in b.instructions:
                lim = maxw
                si = inst.sync_info
                if si is not None and si.on_wait and len(si.on_wait) > lim:
                    waits = list(si.on_wait)
                    chunks = [waits[i:i + lim] for i in range(0, len(waits), lim)]
                    for ci, ch in enumerate(chunks[:-1]):
                        d = mybir.InstNoOp(
                            name=f"{inst.name}-waitsplit{ci}",
                            ins=[], outs=[],
                            sync_info=mybir.SyncInfo(on_wait=ch, on_update=[]),
                        )
                        d.engine = inst.engine
                        nc.register_instruction(d)
                        newlist.append(d)
                    si.on_wait = chunks[-1]
                newlist.append(inst)
            b.instructions = newlist


def build_nc():
    nc = bass.Bass()
    x = nc.dram_tensor("x", [T_CORE, IN_DIM], mybir.dt.float32,
                       kind="ExternalInput")
    # host-pre-laid-out weights: every [P, ...] DMA is contiguous per partition
    w1t = nc.dram_tensor("w1t", [UNITS, K2, P, K1 * P], BF16,
                         kind="ExternalInput")
    w2t = nc.dram_tensor("w2t", [UNITS, NOSL, P, K2 * OSL], BF16,
                         kind="ExternalInput")
    rwt = nc.dram_tensor("rwt", [P, K1, E], mybir.dt.float32,
                         kind="ExternalInput")
    normw = nc.dram_tensor("normw", [P, K1], mybir.dt.float32,
                           kind="ExternalInput")
    b1t = nc.dram_tensor("b1t", [P, UNITS, K2], mybir.dt.float32,
                         kind="ExternalInput")
    b2aug = nc.dram_tensor("b2aug", [P, OUT_DIM], BF16, kind="ExternalInput")
    y = nc.dram_tensor("y", [T_CORE, OUT_DIM], mybir.dt.float32,
                       kind="ExternalOutput")

    f32 = mybir.dt.float32

    with tile.TileContext(nc) as tc:
        with (
            tc.tile_pool(name="singles", bufs=1) as singles,
            tc.tile_pool(name="xq", bufs=2) as xq_pool,
            tc.tile_pool(name="sq", bufs=1) as sq_pool,
            tc.tile_pool(name="xnt", bufs=1) as xnt_pool,
            tc.tile_pool(name="ht", bufs=2) as ht_pool,
            tc.tile_pool(name="oacc", bufs=1) as oacc_pool,
            tc.tile_pool(name="w1s", bufs=3) as w1_pool,
            tc.tile_pool(name="xnf", bufs=3) as xnf_pool,
            tc.tile_pool(name="w2s", bufs=2) as w2_pool,
            tc.tile_pool(name="grow", bufs=2) as grow_pool,
            tc.tile_pool(name="gsmall", bufs=3) as gsmall,
            tc.tile_pool(name="psum1", bufs=2, space="PSUM") as psum1_pool,
            tc.tile_pool(name="psum2", bufs=2, space="PSUM") as psum2_pool,
            tc.tile_pool(name="ptr", bufs=2, space="PSUM") as ptr_pool,
            tc.tile_pool(name="psmall", bufs=1, space="PSUM") as psmall_pool,
        ):
            # ---- constants ----
            ident = singles.tile([P, P], f32)
            make_identity(nc, ident)
            eps_sb = singles.tile([P, 1], f32)
            nc.vector.memset(eps_sb, EPS_NORM)
            normw_sb = singles.tile([P, K1], f32)
            nc.sync.dma_start(normw_sb, normw[:, :])
            rwt_sb = singles.tile([P, K1, E], f32)
            nc.sync.dma_start(rwt_sb, rwt[:, :, :])
            b1_sb = singles.tile([P, UNITS, K2], f32)
            nc.sync.dma_start(b1_sb, b1t[:, :, :])
            b2_sb = singles.tile([P, OUT_DIM], BF16)
            nc.sync.dma_start(b2_sb, b2aug[:, :])
            gates_init = singles.tile([P, NT], BF16)
            nc.vector.memset(gates_init, 0.0)
            nc.vector.memset(gates_init[ONES_ROW:ONES_ROW + 1, :], 1.0)
            # warm-up: keep the PE busy through the initial weight-DMA wait
            # so the HAM clock gate opens before the first real matmul
            for wi in range(96):
                pwarm = psum1_pool.tile([P, NT], f32, tag="ps1", name="pwarm")
                nc.tensor.matmul(
                    pwarm, lhsT=gates_init[:, :P], rhs=gates_init,
                    start=True, stop=True,
                )

            for c in range(NCHUNK):
                c0 = c * NT
                # chunk-resident K-major normalized tokens [P, K1, NT]
                xnT = xnt_pool.tile([P, K1, NT], BF16, tag="xnt")
                # gate rows for the bias matmul (rows 0..3 = expert gates,
                # row 4 = 1.0 for the shared bias, rest 0)
                gates = grow_pool.tile([P, NT], BF16, tag="gates")
                nc.vector.tensor_copy(gates, gates_init)
                # token-major gates for the combine scaling
                gtok = grow_pool.tile([P, TT_PER_CHUNK, E], f32, tag="gtok")

                # ---------- phase A: rmsnorm + transpose + router ----------
                for tt in range(TT_PER_CHUNK):
                    ts = tt * P
                    tok0 = c0 + ts                    # global token row
                    tlen = min(P, max(0, T_CORE - tok0))
                    xq = xq_pool.tile([P, IN_DIM], f32, tag="xq")
                    if tlen < P:
                        nc.vector.memset(xq, 0.0)
                    if tlen > 0:
                        nc.sync.dma_start(xq[:tlen, :], x[tok0:tok0 + tlen, :])
                    # sum of squares via ACT Square with free-accum
                    ssqs = gsmall.tile([P, 5], f32, tag="ssqs")
                    for q in range(5):
                        sq = sq_pool.tile([P, IN_DIM // 5], BF16, tag="sq")
                        nc.scalar.activation(
                            out=sq, in_=xq[:, q * 1024:(q + 1) * 1024],
                            func=mybir.ActivationFunctionType.Square,
                            accum_out=ssqs[:, q:q + 1],
                        )
                    ssq = gsmall.tile([P, 1], f32, tag="ssq")
                    nc.vector.reduce_sum(ssq, ssqs, axis=mybir.AxisListType.X)
                    # rstd = 1/sqrt(ssq/IN_DIM + eps)
                    rstd = gsmall.tile([P, 1], f32, tag="rstd")
                    nc.scalar.activation(
                        out=rstd, in_=ssq,
                        func=mybir.ActivationFunctionType.Sqrt,
                        bias=eps_sb, scale=1.0 / IN_DIM,
                    )
                    nc.vector.reciprocal(rstd, rstd)
                    # diag(rstd) so the transpose matmul applies the scale
                    diag = gsmall.tile([P, P], f32, tag="diag")
                    nc.gpsimd.affine_select(
                        out=diag, in_=rstd.to_broadcast((P, P)),
                        pattern=[[-1, P]], channel_multiplier=1, base=0,
                        compare_op=mybir.AluOpType.is_equal, fill=0.0,
                    )
                    # transpose: xn = norm_w * (xq.T @ diag), staged in f32
                    # for exact router logits, cast to bf16 for the MLPs
                    pr_l = psmall_pool.tile([P, E], f32, tag="ps")
                    for kb in range(K1 // 4):
                        ptr = ptr_pool.tile([P, 4, P], f32, tag="ptr")
                        for kq in range(4):
                            kg = kb * 4 + kq
                            nc.tensor.matmul(
                                ptr[:, kq, :],
                                lhsT=xq[:, kg * P:(kg + 1) * P],
                                rhs=diag,
                                start=True, stop=True,
                            )
                        xnf = xnf_pool.tile([P, 4, P], f32, tag="xnf")
                        nc.vector.tensor_mul(
                            xnf, ptr,
                            normw_sb[:, kb * 4:(kb + 1) * 4].to_broadcast(
                                (P, 4, P)),
                        )
                        nc.scalar.activation(
                            out=xnT[:, kb * 4:(kb + 1) * 4, ts:ts + P],
                            in_=xnf,
                            func=mybir.ActivationFunctionType.Copy)
                        for kq in range(4):
                            kg = kb * 4 + kq
                            nc.tensor.matmul(
                                pr_l,
                                lhsT=xnf[:, kq, :],
                                rhs=rwt_sb[:, kg, :],
                                start=(kg == 0), stop=(kg == K1 - 1),
                            )
                    lg = gsmall.tile([P, E], f32, tag="lg")
                    nc.vector.tensor_copy(lg, pr_l)
                    # softmax over E (free dim)
                    mx = gsmall.tile([P, 1], f32, tag="mx")
                    nc.vector.reduce_max(mx, lg, axis=mybir.AxisListType.X)
                    nmx = gsmall.tile([P, 1], f32, tag="nmx")
                    nc.scalar.mul(nmx, mx, -1.0)
                    pr = gsmall.tile([P, E], f32, tag="pr")
                    nc.scalar.activation(
                        out=pr, in_=lg, func=mybir.ActivationFunctionType.Exp,
                        bias=nmx, scale=1.0,
                    )
                    ssum = gsmall.tile([P, 1], f32, tag="ssum")
                    nc.vector.reduce_sum(ssum, pr, axis=mybir.AxisListType.X)
                    rsum = gsmall.tile([P, 1], f32, tag="rsum")
                    nc.vector.reciprocal(rsum, ssum)
                    nc.vector.tensor_scalar_mul(pr, pr, rsum)
                    # top-2 membership + renormalize
                    m1 = gsmall.tile([P, 1], f32, tag="m1")
                    nc.vector.reduce_max(m1, pr, axis=mybir.AxisListType.X)
                    mk = gsmall.tile([P, E], f32, tag="mk")
                    nc.vector.tensor_scalar(
                        out=mk, in0=pr, scalar1=m1, scalar2=None,
                        op0=mybir.AluOpType.is_equal,
                    )
                    mskd = gsmall.tile([P, E], f32, tag="mskd")
                    nc.vector.tensor_scalar(
                        out=mskd, in0=mk, scalar1=-2.0, scalar2=None,
                        op0=mybir.AluOpType.mult,
                    )
                    nc.vector.tensor_add(mskd, mskd, pr)
                    m2 = gsmall.tile([P, 1], f32, tag="m2")
                    nc.vector.reduce_max(m2, mskd, axis=mybir.AxisListType.X)
                    den = gsmall.tile([P, 1], f32, tag="den")
                    nc.vector.tensor_add(den, m1, m2)
                    nc.vector.tensor_scalar_add(den, den, EPS_GATE)
                    rden = gsmall.tile([P, 1], f32, tag="rden")
                    nc.vector.reciprocal(rden, den)
                    sel = gsmall.tile([P, E], f32, tag="sel")
                    nc.vector.tensor_scalar(
                        out=sel, in0=pr, scalar1=m2, scalar2=None,
                        op0=mybir.AluOpType.is_ge,
                    )
                    nc.vector.tensor_mul(gtok[:, tt, :], pr, sel)
                    nc.vector.tensor_scalar_mul(gtok[:, tt, :], gtok[:, tt, :],
                                                rden)
                    # transpose gates to expert-major rows [E, P] for bias MM
                    pgt = psmall_pool.tile([P, P], f32, tag="pgt")
                    nc.tensor.transpose(pgt[:E, :], gtok[:, tt, :], ident)
                    nc.vector.tensor_copy(gates[0:E, ts:ts + P], pgt[:E, :])

                # ---------- phase B: expert MLPs ----------
                oacc = oacc_pool.tile([P, TT_PER_CHUNK, OUT_DIM], f32,
                                      tag="oacc")
                for u in range(UNITS):
                    hT = ht_pool.tile([P, K2, NT], BF16, tag="ht")
                    for h in range(K2):
                        w1s = w1_pool.tile([P, K1, P], BF16, tag="w1s")
                        nc.sync.dma_start(
                            w1s, w1t[u, h].rearrange("p (k f) -> p k f", f=P))
                        ps1 = psum1_pool.tile([P, NT], f32, tag="ps1")
                        for k in range(K1):
                            nc.tensor.matmul(
                                ps1,
                                lhsT=w1s[:, k, :],
                                rhs=xnT[:, k, :],
                                start=(k == 0), stop=(k == K1 - 1),
                            )
                        nc.scalar.activation(
                            out=hT[:, h, :], in_=ps1,
                            func=mybir.ActivationFunctionType.Gelu,
                            bias=b1_sb[:, u, h:h + 1], scale=1.0,
                        )
                    for osl in range(NOSL):
                        w2s = w2_pool.tile([P, K2, OSL], BF16, tag="w2s")
                        nc.sync.dma_start(
                            w2s, w2t[u, osl].rearrange("p (k f) -> p k f",
                                                       f=OSL))
                        osl_sl = slice(osl * OSL, (osl + 1) * OSL)
                        for tt in range(TT_PER_CHUNK):
                            ts = tt * P
                            ps2 = psum2_pool.tile([P, OSL], f32, tag="ps2")
                            for k in range(K2):
                                nc.tensor.matmul(
                                    ps2,
                                    lhsT=hT[:, k, ts:ts + P],
                                    rhs=w2s[:, k, :],
                                    start=(k == 0),
                                    stop=(k == K2 - 1 and u != UNITS - 1),
                                )
                            if u < E:
                                # scale by this expert's gate (per token row)
                                nc.vector.tensor_scalar_mul(
                                    ps2, ps2, gtok[:, tt, u:u + 1]
                                )
                                if u == 0:
                                    nc.vector.tensor_copy(
                                        oacc[:, tt, osl_sl], ps2
                                    )
                                else:
                                    nc.vector.tensor_add(
                                        oacc[:, tt, osl_sl],
                                        oacc[:, tt, osl_sl], ps2,
                                    )
                            else:
                                # shared unit: fold gate-weighted b2 + sb2
                                nc.tensor.matmul(
                                    ps2,
                                    lhsT=gates[:, ts:ts + P],
                                    rhs=b2_sb[:, osl_sl],
                                    start=False, stop=True,
                                )
                                nc.vector.tensor_add(
                                    oacc[:, tt, osl_sl],
                                    oacc[:, tt, osl_sl], ps2,
                                )
                # ---------- output ----------
                for tt in range(TT_PER_CHUNK):
                    tok0 = c0 + tt * P
                    tlen = min(P, max(0, T_CORE - tok0))
                    if tlen > 0:
                        nc.sync.dma_start(
                            y[tok0:tok0 + tlen, :], oacc[:tlen, tt, :]
                        )

    split_excess_waits(nc)
    return nc


_NC_CACHE = {}


def _get_nc():
    if "nc" not in _NC_CACHE:
        _NC_CACHE["nc"] = build_nc()
    return _NC_CACHE["nc"]


def build_in_maps(x, norm_w, router_w, w1, b1, w2, b2, sw1, sb1, sw2, sb2):
    x = np.ascontiguousarray(np.asarray(x, dtype=np.float32))
    norm_w = np.asarray(norm_w, dtype=np.float32)
    router_w = np.asarray(router_w, dtype=np.float32)

    Bsz, S, D = x.shape          # [16, 1500, 1280]
    out_len = (S - 4) // 4 + 1   # 375
    # frame stacking is a free reshape: [B, S, D] -> [B, out_len, 4*D]
    xs = x[:, :out_len * 4, :].reshape(Bsz, out_len, 4 * D)

    # stack 5 units: 4 experts + shared
    w1_all = np.concatenate([w1, sw1[None]], axis=0)       # [5, HID, IN]
    b1_all = np.concatenate([b1, sb1[None]], axis=0)       # [5, HID]
    w2_all = np.concatenate([w2, sw2[None]], axis=0)       # [5, OUT, HID]

    # router weights K-major: [P, K1, E]
    rwt_np = np.ascontiguousarray(
        router_w.T.reshape(K1, P, E).transpose(1, 0, 2))
    normw_np = np.ascontiguousarray(norm_w.reshape(K1, P).T)
    b2aug_np = np.zeros((P, OUT_DIM), dtype=np.float32)
    b2aug_np[:E, :] = b2 / WS
    b2aug_np[ONES_ROW, :] = sb2 / WS
    b2aug_np = b2aug_np.astype(NPBF16)

    half_maps = []
    for s in range(WS):
        hsl = slice(s * HIDH, (s + 1) * HIDH)
        w1h = w1_all[:, hsl, :]                            # [5, HIDH, IN]
        # [u, h, pi, po, f] = w1h[u, h*P+f, po*P+pi]
        w1t_np = np.ascontiguousarray(
            w1h.reshape(UNITS, K2, P, K1, P).transpose(0, 1, 4, 3, 2)
            .reshape(UNITS, K2, P, K1 * P)).astype(NPBF16)
        w2h = w2_all[:, :, hsl]                            # [5, OUT, HIDH]
        # [u, osl, pi, po, f] = w2h[u, osl*OSL+f, po*P+pi]
        w2t_np = np.ascontiguousarray(
            w2h.reshape(UNITS, NOSL, OSL, K2, P).transpose(0, 1, 4, 3, 2)
            .reshape(UNITS, NOSL, P, K2 * OSL)).astype(NPBF16)
        b1t_np = np.ascontiguousarray(
            b1_all[:, hsl].reshape(UNITS, K2, P).transpose(2, 0, 1))
        half_maps.append({
            "w1t": w1t_np, "w2t": w2t_np, "b1t": b1t_np,
            "rwt": rwt_np, "normw": normw_np, "b2aug": b2aug_np,
        })

    in_maps = []
    for core in range(8):
        g, s = divmod(core, WS)
        xg = np.ascontiguousarray(
            xs[g * B_PER_G:(g + 1) * B_PER_G].reshape(T_CORE, IN_DIM))
        m = dict(half_maps[s])
        m["x"] = xg
        in_maps.append(m)
    return in_maps


def kernel(x, norm_w, router_w, w1, b1, w2, b2, sw1, sb1, sw2, sb2):
    in_maps = build_in_maps(x, norm_w, router_w, w1, b1, w2, b2,
                            sw1, sb1, sw2, sb2)
    nc = _get_nc()
    # the axon-proxied execute occasionally hits a transient
    # NRT_EXEC_UNIT_UNRECOVERABLE; retry a few times
    last_exc = None
    for attempt in range(4):
        try:
            res = run_bass_kernel_spmd(nc, in_maps, core_ids=list(range(8)))
            break
        except Exception as exc:  # noqa: BLE001
            last_exc = exc
            import time
            time.sleep(5)
    else:
        raise last_exc

    Bsz = 16
    out_len = 375
    out = np.empty((Bsz, out_len, OUT_DIM), dtype=np.float32)
    for g in range(TG):
        part = res.results[g * WS]["y"] + res.results[g * WS + 1]["y"]
        out[g * B_PER_G:(g + 1) * B_PER_G] = part.reshape(
            B_PER_G, out_len, OUT_DIM)
    return out


# revision 20
# speedup vs baseline: 1.0144x; 1.0022x over previous
"""MoE audio projector kernel for 8 Trainium2 NeuronCores.

Sharding: 8 cores = 4 token groups x 2 weight (HID) halves.
  - Each token group g covers batches [4g, 4g+4) = 1500 stacked tokens.
  - Each half s computes all 5 MLP units (4 experts + shared) restricted to
    HID rows [s*1024, (s+1)*1024); host sums the two partial outputs.
Everything else (frame stacking = free reshape, RMSNorm, router softmax,
top-2 gates, gelu MLPs, gate-weighted combine, biases) runs on device.

Matmuls run in bf16 (full-rate PE mode) accumulating in fp32 PSUM; weights
are pre-laid-out on host so every DMA is 128 partitions x contiguous bytes.
"""
import sys

sys.path.insert(0, "/opt/trn_rl_repo")

import numpy as np
import ml_dtypes

import concourse.bass as bass
import concourse.mybir as mybir
import concourse.tile as tile
from concourse.bass_utils import run_bass_kernel_spmd
from concourse.masks import make_identity

P = 128
IN_DIM = 5120
K1 = IN_DIM // P          # 40 contraction tiles for mm1 / router
HID = 2048
WS = 2                    # weight-split ways (HID halves)
HIDH = HID // WS          # 1024
K2 = HIDH // P            # 8 contraction tiles for mm2
OUT_DIM = 2048
UNITS = 5                 # 4 experts + shared
E = 4
TG = 4                    # token groups
B_PER_G = 16 // TG        # 4 batches per group
T_CORE = 1500             # valid tokens per core
NT = 512                  # token chunk (SBUF-resident)
NCHUNK = 3                # 3 x 512 = 1536 (36 pad tokens)
TT_PER_CHUNK = NT // P    # 4
OSL = 512                 # mm2 output slice
NOSL = OUT_DIM // OSL     # 4
EPS_NORM = 1e-6
EPS_GATE = 1e-6
ONES_ROW = 32             # partition holding the constant-1 gate row

BF16 = mybir.dt.bfloat16
NPBF16 = ml_dtypes.bfloat16


def split_excess_waits(nc, maxw=1):
    """This container's walrus build only accepts one sync-wait command on
    CTRL-class instructions (Drain) and two on regular ones; fan extra waits
    out onto preceding same-engine NoOps."""
    for f in nc.m.functions:
        for b in f.blocks:
            newlist = []
            for inst in b.instructions:
                lim = maxw
                si = inst.sync_info
                if si is not None and si.on_wait and len(si.on_wait) > lim:
                    waits = list(si.on_wait)
                    chunks = [waits[i:i + lim] for i in range(0, len(waits), lim)]
                    for ci, ch in enumerate(chunks[:-1]):
                        d = mybir.InstNoOp(
                            name=f"{inst.name}-waitsplit{ci}",
                            ins=[], outs=[],
                            sync_info=mybir.SyncInfo(on_wait=ch, on_update=[]),
                        )
                        d.engine = inst.engine
                        nc.register_instruction(d)
                        newlist.append(d)
                    si.on_wait = chunks[-1]
                newlist.append(inst)
            b.instructions = newlist


def build_nc():
    nc = bass.Bass()
    x = nc.dram_tensor("x", [T_CORE, IN_DIM], mybir.dt.float32,
                       kind="ExternalInput")
    # host-pre-laid-out weights: every [P, ...] DMA is contiguous per partition
    w1t = nc.dram_tensor("w1t", [UNITS, K2, P, K1 * P], BF16,
                         kind="ExternalInput")
    w2t = nc.dram_tensor("w2t", [UNITS, NOSL, P, K2 * OSL], BF16,
                         kind="ExternalInput")
    rwt = nc.dram_tensor("rwt", [P, K1, E], mybir.dt.float32,
                         kind="ExternalInput")
    normw = nc.dram_tensor("normw", [P, K1], mybir.dt.float32,
                           kind="ExternalInput")
    b1t = nc.dram_tensor("b1t", [P, UNITS, K2], mybir.dt.float32,
                         kind="ExternalInput")
    b2aug = nc.dram_tensor("b2aug", [P, OUT_DIM], BF16, kind="ExternalInput")
    y = nc.dram_tensor("y", [T_CORE, OUT_DIM], mybir.dt.float32,
                       kind="ExternalOutput")

    f32 = mybir.dt.float32

    with tile.TileContext(nc) as tc:
        with (
            tc.tile_pool(name="singles", bufs=1) as singles,
            tc.tile_pool(name="xq", bufs=2) as xq_pool,
            tc.tile_pool(name="sq", bufs=1) as sq_pool,
            tc.tile_pool(name="xnt", bufs=1) as xnt_pool,
            tc.tile_pool(name="ht", bufs=2) as ht_pool,
            tc.tile_pool(name="oacc", bufs=1) as oacc_pool,
            tc.tile_pool(name="w1s", bufs=3) as w1_pool,
            tc.tile_pool(name="xnf", bufs=3) as xnf_pool,
            tc.tile_pool(name="w2s", bufs=2) as w2_pool,
            tc.tile_pool(name="grow", bufs=2) as grow_pool,
            tc.tile_pool(name="gsmall", bufs=3) as gsmall,
            tc.tile_pool(name="psum1", bufs=2, space="PSUM") as psum1_pool,
            tc.tile_pool(name="psum2", bufs=2, space="PSUM") as psum2_pool,
            tc.tile_pool(name="ptr", bufs=2, space="PSUM") as ptr_pool,
            tc.tile_pool(name="psmall", bufs=1, space="PSUM") as psmall_pool,
        ):
            # ---- constants ----
            ident = singles.tile([P, P], f32)
            make_identity(nc, ident)
            eps_sb = singles.tile([P, 1], f32)
            nc.vector.memset(eps_sb, EPS_NORM)
            normw_sb = singles.tile([P, K1], f32)
            nc.sync.dma_start(normw_sb, normw[:, :])
            rwt_sb = singles.tile([P, K1, E], f32)
            nc.sync.dma_start(rwt_sb, rwt[:, :, :])
            b1_sb = singles.tile([P, UNITS, K2], f32)
            nc.sync.dma_start(b1_sb, b1t[:, :, :])
            b2_sb = singles.tile([P, OUT_DIM], BF16)
            nc.sync.dma_start(b2_sb, b2aug[:, :])
            gates_init = singles.tile([P, NT], BF16)
            nc.vector.memset(gates_init, 0.0)
            nc.vector.memset(gates_init[ONES_ROW:ONES_ROW + 1, :], 1.0)
            # warm-up: keep the PE busy through the initial weight-DMA wait
            # so the HAM clock gate opens before the first real matmul
            for wi in range(96):
                pwarm = psum1_pool.tile([P, NT], f32, tag="ps1", name="pwarm")
                nc.tensor.matmul(
                    pwarm, lhsT=gates_init[:, :P], rhs=gates_init,
                    start=True, stop=True,
                )

            for c in range(NCHUNK):
                c0 = c * NT
                # chunk-resident K-major normalized tokens [P, K1, NT]
                xnT = xnt_pool.tile([P, K1, NT], BF16, tag="xnt")
                # gate rows for the bias matmul (rows 0..3 = expert gates,
                # row 4 = 1.0 for the shared bias, rest 0)
                gates = grow_pool.tile([P, NT], BF16, tag="gates")
                nc.vector.tensor_copy(gates, gates_init)
                # token-major gates for the combine scaling
                gtok = grow_pool.tile([P, TT_PER_CHUNK, E], f32, tag="gtok")

                # ---------- phase A: rmsnorm + transpose + router ----------
                for tt in range(TT_PER_CHUNK):
                    ts = tt * P
                    tok0 = c0 + ts                    # global token row
                    tlen = min(P, max(0, T_CORE - tok0))
                    xq = xq_pool.tile([P, IN_DIM], f32, tag="xq")
                    if tlen < P:
                        nc.vector.memset(xq, 0.0)
                    if tlen > 0:
                        nc.sync.dma_start(xq[:tlen, :], x[tok0:tok0 + tlen, :])
                    # sum of squares via ACT Square with free-accum
                    ssqs = gsmall.tile([P, 5], f32, tag="ssqs")
                    for q in range(5):
                        sq = sq_pool.tile([P, IN_DIM // 5], BF16, tag="sq")
                        nc.scalar.activation(
                            out=sq, in_=xq[:, q * 1024:(q + 1) * 1024],
                            func=mybir.ActivationFunctionType.Square,
                            accum_out=ssqs[:, q:q + 1],
                        )
                    ssq = gsmall.tile([P, 1], f32, tag="ssq")
                    nc.vector.reduce_sum(ssq, ssqs, axis=mybir.AxisListType.X)
                    # rstd = 1/sqrt(ssq/IN_DIM + eps)
                    rstd = gsmall.tile([P, 1], f32, tag="rstd")
                    nc.scalar.activation(
                        out=rstd, in_=ssq,
                        func=mybir.ActivationFunctionType.Sqrt,
                        bias=eps_sb, scale=1.0 / IN_DIM,
                    )
                    nc.vector.reciprocal(rstd, rstd)
                    # diag(rstd) so the transpose matmul applies the scale
                    diag = gsmall.tile([P, P], f32, tag="diag")
                    nc.gpsimd.affine_select(
                        out=diag, in_=rstd.to_broadcast((P, P)),
                        pattern=[[-1, P]], channel_multiplier=1, base=0,
                        compare_op=mybir.AluOpType.is_equal, fill=0.0,
                    )
                    # transpose: xn = norm_w * (xq.T @ diag), staged in f32
                    # for exact router logits, cast to bf16 for the MLPs
                    pr_l = psmall_pool.tile([P, E], f32, tag="ps")
                    for kb in range(K1 // 4):
                        ptr = ptr_pool.tile([P, 4, P], f32, tag="ptr")
                        for kq in range(4):
                            kg = kb * 4 + kq
                            nc.tensor.matmul(
                                ptr[:, kq, :],
                                lhsT=xq[:, kg * P:(kg + 1) * P],
                                rhs=diag,
                                start=True, stop=True,
                            )
                        xnf = xnf_pool.tile([P, 4, P], f32, tag="xnf")
                        nc.vector.tensor_mul(
                            xnf, ptr,
                            normw_sb[:, kb * 4:(kb + 1) * 4].to_broadcast(
                                (P, 4, P)),
                        )
                        nc.scalar.activation(
                            out=xnT[:, kb * 4:(kb + 1) * 4, ts:ts + P],
                            in_=xnf,
                            func=mybir.ActivationFunctionType.Copy)
                        for kq in range(4):
                            kg = kb * 4 + kq
                            nc.tensor.matmul(
                                pr_l,
                                lhsT=xnf[:, kq, :],
                                rhs=rwt_sb[:, kg, :],
                                start=(kg == 0), stop=(kg == K1 - 1),
                            )
                    lg = gsmall.tile([P, E], f32, tag="lg")
                    nc.vector.tensor_copy(lg, pr_l)
                    # softmax over E (free dim)
                    mx = gsmall.tile([P, 1], f32, tag="mx")
                    nc.vector.reduce_max(mx, lg, axis=mybir.AxisListType.X)
                    nmx = gsmall.tile([P, 1], f32, tag="nmx")
                    nc.scalar.mul(nmx, mx, -1.0)
                    pr = gsmall.tile([P, E], f32, tag="pr")
                    nc.scalar.activation(
                        out=pr, in_=lg, func=mybir.ActivationFunctionType.Exp,
                        bias=nmx, scale=1.0,
                    )
                    ssum = gsmall.tile([P, 1], f32, tag="ssum")
                    nc.vector.reduce_sum(ssum, pr, axis=mybir.AxisListType.X)
                    rsum = gsmall.tile([P, 1], f32, tag="rsum")
                    nc.vector.reciprocal(rsum, ssum)
                    nc.vector.tensor_scalar_mul(pr, pr, rsum)
                    # top-2 membership + renormalize
                    m1 = gsmall.tile([P, 1], f32, tag="m1")
                    nc.vector.reduce_max(m1, pr, axis=mybir.AxisListType.X)
                    mk = gsmall.tile([P, E], f32, tag="mk")
                    nc.vector.tensor_scalar(
                        out=mk, in0=pr, scalar1=m1, scalar2=None,
                        op0=mybir.AluOpType.is_equal,
                    )
                    mskd = gsmall.tile([P, E], f32, tag="mskd")
                    nc.vector.tensor_scalar(
                        out=mskd, in0=mk, scalar1=-2.0, scalar2=None,
                        op0=mybir.AluOpType.mult,
                    )
                    nc.vector.tensor_add(mskd, mskd, pr)
                    m2 = gsmall.tile([P, 1], f32, tag="m2")
                    nc.vector.reduce_max(m2, mskd, axis=mybir.AxisListType.X)
                    den = gsmall.tile([P, 1], f32, tag="den")
                    nc.vector.tensor_add(den, m1, m2)
                    nc.vector.tensor_scalar_add(den, den, EPS_GATE)
                    rden = gsmall.tile([P, 1], f32, tag="rden")
                    nc.vector.reciprocal(rden, den)
                    sel = gsmall.tile([P, E], f32, tag="sel")
                    nc.vector.tensor_scalar(
                        out=sel, in0=pr, scalar1=m2, scalar2=None,
                        op0=mybir.AluOpType.is_ge,
                    )
                    nc.vector.tensor_mul(gtok[:, tt, :], pr, sel)
                    nc.vector.tensor_scalar_mul(gtok[:, tt, :], gtok[:, tt, :],
                                                rden)
                    # transpose gates to expert-major rows [E, P] for bias MM
                    pgt = psmall_pool.tile([P, P], f32, tag="pgt")
                    nc.tensor.transpose(pgt[:E, :], gtok[:, tt, :], ident)
                    nc.vector.tensor_copy(gates[0:E, ts:ts + P], pgt[:E, :])

                # ---------- phase B: expert MLPs ----------
                oacc = oacc_pool.tile([P, TT_PER_CHUNK, OUT_DIM], f32,
                                      tag="oacc")
                for u in range(UNITS):
                    hT = ht_pool.tile([P, K2, NT], BF16, tag="ht")
                    for h in range(K2):
                        w1s = w1_pool.tile([P, K1, P], BF16, tag="w1s")
                        nc.sync.dma_start(
                            w1s, w1t[u, h].rearrange("p (k f) -> p k f", f=P))
                        ps1 = psum1_pool.tile([P, NT], f32, tag="ps1")
                        for k in range(K1):
                            nc.tensor.matmul(
                                ps1,
                                lhsT=w1s[:, k, :],
                                rhs=xnT[:, k, :],
                                start=(k == 0), stop=(k == K1 - 1),
                            )
                        nc.scalar.activation(
                            out=hT[:, h, :], in_=ps1,
                            func=mybir.ActivationFunctionType.Gelu,
                            bias=b1_sb[:, u, h:h + 1], scale=1.0,
                        )
                    for osl in range(NOSL):
                        w2s = w2_pool.tile([P, K2, OSL], BF16, tag="w2s")
                        nc.sync.dma_start(
                            w2s, w2t[u, osl].rearrange("p (k f) -> p k f",
                                                       f=OSL))
                        osl_sl = slice(osl * OSL, (osl + 1) * OSL)
                        for tt in range(TT_PER_CHUNK):
                            ts = tt * P
                            ps2 = psum2_pool.tile([P, OSL], f32, tag="ps2")
                            for k in range(K2):
                                nc.tensor.matmul(
                                    ps2,
                                    lhsT=hT[:, k, ts:ts + P],
                                    rhs=w2s[:, k, :],
                                    start=(k == 0),
                                    stop=(k == K2 - 1 and u != UNITS - 1),
                                )
                            if u < E:
                                # scale by this expert's gate (per token row)
                                nc.vector.tensor_scalar_mul(
                                    ps2, ps2, gtok[:, tt, u:u + 1]
                                )
                                if u == 0:
                                    nc.vector.tensor_copy(
                                        oacc[:, tt, osl_sl], ps2
                                    )
                                else:
                                    nc.vector.tensor_add(
                                        oacc[:, tt, osl_sl],
                                        oacc[:, tt, osl_sl], ps2,
                                    )
                            else:
                                # shared unit: fold gate-weighted b2 + sb2
                                nc.tensor.matmul(
                                    ps2,
                                    lhsT=gates[:, ts:ts + P],
                                    rhs=b2_sb[:, osl_sl],
                                    start=False, stop=True,
                                )
                                nc.vector.tensor_add(
                                    oacc[:, tt, osl_sl],
                                    oacc[:, tt, osl_sl], ps2,
                                )
                # ---------- output ----------
                for tt in range(TT_PER_CHUNK):
                    tok0 = c0 + tt * P
                    tlen = min(P, max(0, T_CORE - tok0))
                    if tlen > 0:
                        nc.sync.dma_start(
                            y[tok0:tok0 + tlen, :], oacc[:tlen, tt, :]
                        )

    split_excess_waits(nc)
    return nc


_NC_CACHE = {}


def _get_nc():
    if "nc" not in _NC_CACHE:
        _NC_CACHE["nc"] = build_nc()
    return _NC_CACHE["nc"]


def build_in_maps(x, norm_w, router_w, w1, b1, w2, b2, sw1, sb1, sw2, sb2):
    x = np.ascontiguousarray(np.asarray(x, dtype=np.float32))
    norm_w = np.asarray(norm_w, dtype=np.float32)
    router_w = np.asarray(router_w, dtype=np.float32)

    Bsz, S, D = x.shape          # [16, 1500, 1280]
    out_len = (S - 4) // 4 + 1   # 375
    # frame stacking is a free reshape: [B, S, D] -> [B, out_len, 4*D]
    xs = x[:, :out_len * 4, :].reshape(Bsz, out_len, 4 * D)

    # stack 5 units: 4 experts + shared
    w1_all = np.concatenate([w1, sw1[None]], axis=0)       # [5, HID, IN]
    b1_all = np.concatenate([b1, sb1[None]], axis=0)       # [5, HID]
    w2_all = np.concatenate([w2, sw2[None]], axis=0)       # [5, OUT, HID]

    # router weights K-major: [P, K1, E]
    rwt_np = np.ascontiguousarray(
        router_w.T.reshape(K1, P, E).transpose(1, 0, 2))
    normw_np = np.ascontiguousarray(norm_w.reshape(K1, P).T)
    b2aug_np = np.zeros((P, OUT_DIM), dtype=np.float32)
    b2aug_np[:E, :] = b2 / WS
    b2aug_np[ONES_ROW, :] = sb2 / WS
    b2aug_np = b2aug_np.astype(NPBF16)

    half_maps = []
    for s in range(WS):
        hsl = slice(s * HIDH, (s + 1) * HIDH)
        w1h = w1_all[:, hsl, :]                            # [5, HIDH, IN]
        # [u, h, pi, po, f] = w1h[u, h*P+f, po*P+pi]
        w1t_np = np.ascontiguousarray(
            w1h.reshape(UNITS, K2, P, K1, P).transpose(0, 1, 4, 3, 2)
            .reshape(UNITS, K2, P, K1 * P)).astype(NPBF16)
        w2h = w2_all[:, :, hsl]                            # [5, OUT, HIDH]
        # [u, osl, pi, po, f] = w2h[u, osl*OSL+f, po*P+pi]
        w2t_np = np.ascontiguousarray(
            w2h.reshape(UNITS, NOSL, OSL, K2, P).transpose(0, 1, 4, 3, 2)
            .reshape(UNITS, NOSL, P, K2 * OSL)).astype(NPBF16)
        b1t_np = np.ascontiguousarray(
            b1_all[:, hsl].reshape(UNITS, K2, P).transpose(2, 0, 1))
        half_maps.append({
            "w1t": w1t_np, "w2t": w2t_np, "b1t": b1t_np,
            "rwt": rwt_np, "normw": normw_np, "b2aug": b2aug_np,
        })

    in_maps = []
    for core in range(8):
        g, s = divmod(core, WS)
        xg = np.ascontiguousarray(
            xs[g * B_PER_G:(g + 1) * B_PER_G].reshape(T_CORE, IN_DIM))
        m = dict(half_maps[s])
        m["x"] = xg
        in_maps.append(m)
    return in_maps


def kernel(x, norm_w, router_w, w1, b1, w2, b2, sw1, sb1, sw2, sb2):
    in_maps = build_in_maps(x, norm_w, router_w, w1, b1, w2, b2,
                            sw1, sb1, sw2, sb2)
    nc = _get_nc()
    # the axon-proxied execute occasionally hits a transient
    # NRT_EXEC_UNIT_UNRECOVERABLE; retry a few times
    last_exc = None
    for attempt in range(4):
        try:
            res = run_bass_kernel_spmd(nc, in_maps, core_ids=list(range(8)))
            break
        except Exception as exc:  # noqa: BLE001
            last_exc = exc
            import time
            time.sleep(5)
    else:
        raise last_exc

    Bsz = 16
    out_len = 375
    out = np.empty((Bsz, out_len, OUT_DIM), dtype=np.float32)
    for g in range(TG):
        part = res.results[g * WS]["y"] + res.results[g * WS + 1]["y"]
        out[g * B_PER_G:(g + 1) * B_PER_G] = part.reshape(
            B_PER_G, out_len, OUT_DIM)
    return out
